# revision 5
# baseline (speedup 1.0000x reference)
"""2-layer GCN (PyG GCNConv x2 + log_softmax) on 8 Trainium2 NeuronCores.

Strategy (1D node partition, edges owned by dst):
  - 100000 nodes -> 8 shards of 12500 (padded to 12544 = 98*128 blocks).
  - Feature table xws[v] = (x[v] @ W1) * dinv[v] is built IN FULL on every
    core (dense matmul from a host-pre-transposed tiled copy of x), written
    to a DRAM table with owner-remapped rows (12544*owner + local).
  - Per-edge messages are fetched with the GPSIMD dma_gather extended
    instruction (int16 indices => table split in 4 sections of 25088 rows,
    one SWDGE queue per section for parallel descriptor generation).
  - Segment-sum onto destination nodes via one-hot matmul: for each block of
    128 dst nodes, S[e, n] = (dst_local[e] == n) built with a broadcast
    is_equal; psum += S^T @ msg on the tensor engine.
  - h = relu(dinv * agg + b1); hws = (h @ W2) * dinv computed per block,
    AllGathered across cores ([100352, 64] table), then layer 2 repeats the
    same gather/aggregate with elem 64 and finishes with a row-wise
    log_softmax over the 47 real classes.

Edge bucketing: edges sorted by (dst block, src section); per-(block,section)
capacities are padded to a multiple of 32 and maxed across cores so all 8
cores run the same program; pad slots gather a guaranteed-zero table row and
carry dst_local = -1 (one-hot row of zeros).
"""

import numpy as np
import ml_dtypes
from contextlib import ExitStack

import concourse.bass as bass
import concourse.bacc as bacc
import concourse.mybir as mybir
import concourse.tile as tile
from concourse.bass_utils import run_bass_kernel_spmd

P = 128
N = 100000
F_IN = 256
HID = 128
NCLS = 47
CPAD = 128           # classes padded so bf16 rows are 256B
NCORES = 8
SHARD = 12500
PSHARD = 12544       # 98 * 128
NBLK = PSHARD // P   # 98
NT = NCORES * NBLK   # 784 table row-tiles
V = NCORES * PSHARD  # 100352 table rows
NSEC = 4
SECROWS = V // NSEC  # 25088 (= 2 owners per section)
ZROW_REL = SHARD     # rel row 12500 inside a section: guaranteed-zero row
NQ = 4               # SWDGE queues; section r uses queue r
GRP = 4              # phase-A table tiles per DMA group
F32 = mybir.dt.float32
BF16 = mybir.dt.bfloat16
DT = BF16             # compute/table dtype: BF16 (fast) or F32 (exact)
DT_NP = np.float32    # host arrays stay f32; device casts on load where needed
NEG_BIG = -1.0e30

_CACHE = {}


def _preprocess(edge_index):
    """Bucket edges by (owner, dst block, src section). Returns per-core
    device arrays + the shared capacity structure."""
    src = edge_index[0].astype(np.int64)
    dst = edge_index[1].astype(np.int64)
    loops = np.arange(N, dtype=np.int64)
    src = np.concatenate([src, loops])
    dst = np.concatenate([dst, loops])

    deg = np.bincount(dst, minlength=N).astype(np.float32)

    row = PSHARD * (src // SHARD) + (src % SHARD)
    rel = (row % SECROWS).astype(np.int64)
    sec = (row // SECROWS).astype(np.int64)
    owner = dst // SHARD
    dst_local = dst % SHARD
    blk = dst_local // P
    wloc = (dst_local % P).astype(np.int64)

    # counts[c, b, r]
    key = (owner * NBLK + blk) * NSEC + sec
    counts = np.bincount(key, minlength=NCORES * NBLK * NSEC).reshape(
        NCORES, NBLK, NSEC)
    caps = counts.max(axis=0)                       # [NBLK, NSEC]
    caps = ((caps + 31) // 32) * 32                 # multiple of 32 (and 16)
    chunks = (caps + P - 1) // P                    # [NBLK, NSEC]

    idx_cols = int(caps.sum() // 16)
    chunk_cols = int(chunks.sum())

    idx_col_start = np.zeros((NBLK, NSEC), np.int64)
    chunk_col_start = np.zeros((NBLK, NSEC), np.int64)
    ic = cc = 0
    for b in range(NBLK):
        for r in range(NSEC):
            idx_col_start[b, r] = ic
            chunk_col_start[b, r] = cc
            ic += caps[b, r] // 16
            cc += chunks[b, r]

    per_core = []
    order_key = key  # sort edges per core by (block, section)
    for c in range(NCORES):
        m = owner == c
        rel_c, sec_c, blk_c, wloc_c = rel[m], sec[m], blk[m], wloc[m]
        o = np.lexsort((sec_c, blk_c))
        rel_c, sec_c, blk_c, wloc_c = rel_c[o], sec_c[o], blk_c[o], wloc_c[o]
        cnt_c = np.bincount((blk_c * NSEC + sec_c),
                            minlength=NBLK * NSEC).reshape(NBLK, NSEC)

        idx16 = np.zeros((P, idx_cols), np.int16)
        dstloc = np.full((P, chunk_cols), -1.0, np.float32)
        pos = 0
        for b in range(NBLK):
            for r in range(NSEC):
                n = int(cnt_c[b, r])
                cap = int(caps[b, r])
                ch = int(chunks[b, r])
                if cap == 0:
                    continue
                relpad = np.full(cap, ZROW_REL, np.int64)
                relpad[:n] = rel_c[pos:pos + n]
                wpad = np.full(ch * P, -1.0, np.float32)
                wpad[:n] = wloc_c[pos:pos + n]
                pos += n
                ics = int(idx_col_start[b, r])
                ccs = int(chunk_col_start[b, r])
                blkidx = relpad.reshape(cap // 16, 16).T.astype(np.int16)
                idx16[:, ics:ics + cap // 16] = np.tile(blkidx, (8, 1))
                dstloc[:, ccs:ccs + ch] = wpad.reshape(ch, P).T
        assert pos == len(rel_c)
        per_core.append((idx16, dstloc))

    meta = dict(caps=caps, chunks=chunks, idx_col_start=idx_col_start,
                chunk_col_start=chunk_col_start, idx_cols=idx_cols,
                chunk_cols=chunk_cols)
    return per_core, deg, meta


def _host_tensors(x, deg, W1, b1, W2, b2):
    """Dense per-core inputs (identical on all cores except none here)."""
    # x pre-transposed + tiled in table-row order:
    # xt[T, k, a, b] = x[node(T, b), 128k + a]
    xt = np.zeros((NT, 2, P, P), np.float32)
    deg_tbl = np.ones((P, NT), np.float32)
    for o in range(NCORES):
        base = o * SHARD
        for u in range(NBLK):
            T = o * NBLK + u
            n0 = base + u * P
            nn = min(P, base + SHARD - n0)
            if nn <= 0:
                continue
            blkx = x[n0:n0 + nn, :]                       # [nn, 256]
            t = blkx.reshape(nn, 2, P).transpose(1, 2, 0)  # [2, 128a, nn]
            xt[T, :, :, :nn] = t
            deg_tbl[:nn, T] = deg[n0:n0 + nn]
    w2p = np.zeros((HID, CPAD), np.float32)
    w2p[:, :NCLS] = W2
    b2p = np.full((1, CPAD), NEG_BIG, np.float32)
    b2p[0, :NCLS] = b2
    return xt, deg_tbl, w2p, b2p


def _deg_own(deg, c):
    d = np.ones((P, NBLK), np.float32)
    sh = deg[c * SHARD:(c + 1) * SHARD]
    full = np.ones(PSHARD, np.float32)
    full[:SHARD] = sh
    return np.ascontiguousarray(full.reshape(NBLK, P).T)


def _build_program(meta):
    caps = meta["caps"]
    chunks = meta["chunks"]
    idx_col_start = meta["idx_col_start"]
    chunk_col_start = meta["chunk_col_start"]
    idx_cols = meta["idx_cols"]
    chunk_cols = meta["chunk_cols"]
    kb = [int(chunks[b].sum()) for b in range(NBLK)]
    kb_max = max(kb)
    ch_max = [int(chunks[:, r].max()) for r in range(NSEC)]

    nc = bacc.Bacc("TRN2", target_bir_lowering=False, debug=False,
                   num_swdge_queues=NQ)

    xt_in = nc.declare_dram_parameter("xt", [NT, 2, P, P], DT, isOutput=False)
    w1_in = nc.declare_dram_parameter("w1", [2, P, HID], DT, isOutput=False)
    b1_in = nc.declare_dram_parameter("b1", [1, HID], F32, isOutput=False)
    w2_in = nc.declare_dram_parameter("w2", [HID, CPAD], DT, isOutput=False)
    b2_in = nc.declare_dram_parameter("b2", [1, CPAD], F32, isOutput=False)
    degt_in = nc.declare_dram_parameter("deg_tbl", [P, NT], F32,
                                        isOutput=False)
    dego_in = nc.declare_dram_parameter("deg_own", [P, NBLK], F32,
                                        isOutput=False)
    idx_in = nc.declare_dram_parameter("idx16", [P, idx_cols],
                                       mybir.dt.int16, isOutput=False)
    dl_in = nc.declare_dram_parameter("dstloc", [P, chunk_cols], DT,
                                      isOutput=False)
    out_par = nc.declare_dram_parameter("out", [PSHARD, NCLS], F32,
                                        isOutput=True)

    table = nc.dram_tensor("xws_table", [V, HID], DT)
    hws_shard = nc.dram_tensor("hws_shard", [PSHARD, CPAD], DT)
    hws_full = nc.dram_tensor("hws_full", [V, CPAD], DT, addr_space="Shared")

    with tile.TileContext(nc) as tc, ExitStack() as ctx:
        const = ctx.enter_context(tc.tile_pool(name="const", bufs=1))
        xload = ctx.enter_context(tc.tile_pool(name="xload", bufs=3))
        xwr = ctx.enter_context(tc.tile_pool(name="xwr", bufs=3))
        msgp = ctx.enter_context(tc.tile_pool(name="msgp", bufs=2))
        msg2p = ctx.enter_context(tc.tile_pool(name="msg2p", bufs=2))
        sp = ctx.enter_context(tc.tile_pool(name="sp", bufs=2))
        ep = ctx.enter_context(tc.tile_pool(name="ep", bufs=3))
        meta_p = ctx.enter_context(tc.tile_pool(name="metap", bufs=2))
        ps_a = ctx.enter_context(tc.tile_pool(name="ps_a", bufs=2,
                                              space="PSUM"))
        ps_g = ctx.enter_context(tc.tile_pool(name="ps_g", bufs=2,
                                              space="PSUM"))
        ps_t = ctx.enter_context(tc.tile_pool(name="ps_t", bufs=2,
                                              space="PSUM"))
        ps_m = ctx.enter_context(tc.tile_pool(name="ps_m", bufs=2,
                                              space="PSUM"))

        # ---------- constants ----------
        w1_sb = const.tile([P, 2, HID], DT)
        nc.sync.dma_start(w1_sb[:], w1_in.ap().rearrange("k a f -> a k f"))
        w2_sb = const.tile([P, CPAD], DT)
        nc.sync.dma_start(w2_sb[:], w2_in.ap())

        b1_row = const.tile([1, HID], F32)
        nc.sync.dma_start(b1_row[:], b1_in.ap())
        b1_rep = const.tile([P, HID], F32)
        nc.gpsimd.partition_broadcast(b1_rep[:], b1_row[:])
        b2_row = const.tile([1, CPAD], F32)
        nc.sync.dma_start(b2_row[:], b2_in.ap())
        b2_rep = const.tile([P, CPAD], F32)
        nc.gpsimd.partition_broadcast(b2_rep[:], b2_row[:])

        iota_i = const.tile([P, P], mybir.dt.int32)
        nc.gpsimd.iota(iota_i[:], pattern=[[1, P]], base=0,
                       channel_multiplier=0)
        iota_f = const.tile([P, P], DT)
        nc.vector.tensor_copy(iota_f[:], iota_i[:])

        ident = const.tile([P, P], DT)
        from concourse.masks import make_identity
        make_identity(nc, ident[:])

        degt_sb = const.tile([P, NT], F32)
        nc.sync.dma_start(degt_sb[:], degt_in.ap())
        dinv_tbl = const.tile([P, NT], F32)
        nc.scalar.sqrt(dinv_tbl[:], degt_sb[:])
        nc.vector.reciprocal(dinv_tbl[:], dinv_tbl[:])

        dego_sb = const.tile([P, NBLK], F32)
        nc.sync.dma_start(dego_sb[:], dego_in.ap())
        dinv_own = const.tile([P, NBLK], F32)
        nc.scalar.sqrt(dinv_own[:], dego_sb[:])
        nc.vector.reciprocal(dinv_own[:], dinv_own[:])

        hws_sb = const.tile([P, NBLK, CPAD], DT)
        out_sb = const.tile([P, NBLK, NCLS], F32)

        # pre-zero gather landing pools (pad-tail slots must be finite)
        for _ in range(2):
            for r in range(NSEC):
                mz = msgp.tile([P, ch_max[r], HID], DT, tag=f"msg{r}")
                nc.vector.memset(mz[:], 0.0)
                mz2 = msg2p.tile([P, ch_max[r], CPAD], DT, tag=f"msg2_{r}")
                nc.vector.memset(mz2[:], 0.0)

        # ---------- phase A: build xws table ----------
        tbl_ap = table.ap()
        for g in range(NT // GRP):
            xsb = xload.tile([P, GRP, 2, P], DT, tag="xsb")
            nc.sync.dma_start(
                xsb[:],
                xt_in.ap()[g * GRP:(g + 1) * GRP].rearrange(
                    "t k a b -> a t k b"))
            xw = xwr.tile([P, GRP, HID], DT, tag="xw")
            for ti in range(GRP):
                T = g * GRP + ti
                psa = ps_a.tile([P, HID], F32, space="PSUM", tag="psa")
                for kc in range(2):
                    nc.tensor.matmul(psa[:], lhsT=xsb[:, ti, kc, :],
                                     rhs=w1_sb[:, kc, :],
                                     start=(kc == 0), stop=(kc == 1))
                nc.vector.tensor_scalar(
                    out=xw[:, ti, :], in0=psa[:],
                    scalar1=dinv_tbl[:, T:T + 1], scalar2=None,
                    op0=mybir.AluOpType.mult)
            nc.sync.dma_start(
                tbl_ap[g * GRP * P:(g + 1) * GRP * P, :].rearrange(
                    "(t p) f -> p t f", p=P),
                xw[:])

        # ---------- phase B: layer-1 gather + aggregate ----------
        def gather_layer(b, elem, src_ap, pool, tagp):
            tiles = []
            for r in range(NSEC):
                cap = int(caps[b, r])
                ch = int(chunks[b, r])
                if cap == 0:
                    tiles.append(None)
                    continue
                mt = pool.tile([P, ch_max[r], elem], DT, tag=f"{tagp}{r}")
                ics = int(idx_col_start[b, r])
                nc.gpsimd.dma_gather(
                    out_ap=mt[:, :ch, :],
                    in_ap=src_ap[r * SECROWS:(r + 1) * SECROWS, :],
                    idxs_ap=idx_sb[:, ics:ics + cap // 16],
                    num_idxs=cap,
                    num_idxs_reg=cap,
                    elem_size=elem,
                    single_packet=(cap <= 1024),
                    queue_num=r,
                )
                tiles.append(mt)
            return tiles

        def build_s(b):
            ccs0 = int(chunk_col_start[b, 0])
            kbb = kb[b]
            s_t = sp.tile([P, kb_max, P], DT, tag="S")
            nc.vector.tensor_tensor(
                out=s_t[:, :kbb, :],
                in0=iota_f[:].rearrange("p w -> p () w").to_broadcast(
                    [P, kbb, P]),
                in1=dl_sb[:, ccs0:ccs0 + kbb].rearrange(
                    "p k -> p k ()").to_broadcast([P, kbb, P]),
                op=mybir.AluOpType.is_equal)
            return s_t

        def aggregate(b, s_t, tiles, elem, ps_pool, tag):
            ps = ps_pool.tile([P, elem], F32, space="PSUM", tag=tag)
            jj = 0
            kbb = kb[b]
            for r in range(NSEC):
                if tiles[r] is None:
                    continue
                for j in range(int(chunks[b, r])):
                    nc.tensor.matmul(ps[:], lhsT=s_t[:, jj, :],
                                     rhs=tiles[r][:, j, :],
                                     start=(jj == 0), stop=(jj == kbb - 1))
                    jj += 1
            assert jj == kbb
            return ps

        idx_sb = const.tile([P, idx_cols], mybir.dt.int16)
        nc.sync.dma_start(idx_sb[:], idx_in.ap())
        dl_sb = const.tile([P, chunk_cols], DT)
        nc.sync.dma_start(dl_sb[:], dl_in.ap())

        for b in range(NBLK):
            tiles = gather_layer(b, HID, tbl_ap, msgp, "msg")
            s_t = build_s(b)
            ps = aggregate(b, s_t, tiles, HID, ps_g, "agg")
            # h = relu(dinv * ps + b1)
            t1 = ep.tile([P, HID], F32, tag="t1")
            nc.vector.tensor_scalar(out=t1[:], in0=ps[:],
                                    scalar1=dinv_own[:, b:b + 1],
                                    scalar2=None, op0=mybir.AluOpType.mult)
            t2 = ep.tile([P, HID], F32, tag="t2")
            nc.vector.tensor_tensor(out=t2[:], in0=t1[:], in1=b1_rep[:],
                                    op=mybir.AluOpType.add)
            h_t = ep.tile([P, HID], DT, tag="h")
            nc.scalar.activation(h_t[:], t2[:],
                                 mybir.ActivationFunctionType.Relu)
            # hws block = (h @ W2) * dinv
            pst = ps_t.tile([P, HID], DT, space="PSUM", tag="tr")
            nc.tensor.transpose(pst[:], h_t[:], ident[:])
            ht_sb = ep.tile([P, HID], DT, tag="ht")
            nc.vector.tensor_copy(ht_sb[:], pst[:])
            psm = ps_m.tile([P, CPAD], F32, space="PSUM", tag="mm2")
            nc.tensor.matmul(psm[:], lhsT=ht_sb[:], rhs=w2_sb[:],
                             start=True, stop=True)
            nc.vector.tensor_scalar(out=hws_sb[:, b, :], in0=psm[:],
                                    scalar1=dinv_own[:, b:b + 1],
                                    scalar2=None, op0=mybir.AluOpType.mult)

        nc.sync.dma_start(
            hws_shard.ap().rearrange("(u p) c -> p u c", p=P), hws_sb[:])
        nc.gpsimd.collective_compute(
            "AllGather",
            mybir.AluOpType.bypass,
            replica_groups=[list(range(NCORES))],
            ins=[hws_shard.ap()],
            outs=[hws_full.ap()],
        )

        # ---------- phase C: layer-2 gather + aggregate + log_softmax ------
        hws_ap = hws_full.ap()
        for b in range(NBLK):
            tiles = gather_layer(b, CPAD, hws_ap, msg2p, "msg2_")
            s_t = build_s(b)
            ps = aggregate(b, s_t, tiles, CPAD, ps_g, "agg")
            t1 = ep.tile([P, CPAD], F32, tag="c1")
            nc.vector.tensor_scalar(out=t1[:], in0=ps[:],
                                    scalar1=dinv_own[:, b:b + 1],
                                    scalar2=None, op0=mybir.AluOpType.mult)
            logit = ep.tile([P, CPAD], F32, tag="c2")
            nc.vector.tensor_tensor(out=logit[:], in0=t1[:], in1=b2_rep[:],
                                    op=mybir.AluOpType.add)
            rmax = ep.tile([P, 1], F32, tag="rmax")
            nc.vector.reduce_max(rmax[:], logit[:, :NCLS],
                                 axis=mybir.AxisListType.X)
            xm = ep.tile([P, NCLS], F32, tag="xm")
            nc.vector.tensor_scalar(out=xm[:], in0=logit[:, :NCLS],
                                    scalar1=rmax[:], scalar2=None,
                                    op0=mybir.AluOpType.subtract)
            et = ep.tile([P, NCLS], F32, tag="et")
            ssum = ep.tile([P, 1], F32, tag="ssum")
            nc.scalar.activation(et[:], xm[:],
                                 mybir.ActivationFunctionType.Exp,
                                 accum_out=ssum[:])
            lse = ep.tile([P, 1], F32, tag="lse")
            nc.scalar.activation(lse[:], ssum[:],
                                 mybir.ActivationFunctionType.Ln)
            nc.vector.tensor_scalar(out=out_sb[:, b, :], in0=xm[:],
                                    scalar1=lse[:], scalar2=None,
                                    op0=mybir.AluOpType.subtract)

        nc.sync.dma_start(
            out_par.ap().rearrange("(u p) c -> p u c", p=P), out_sb[:])

    nc.compile()
    return nc


def kernel(x=None, edge_index=None, W1=None, b1=None, W2=None, b2=None,
           _trace=False):
    x = np.asarray(x, np.float32)
    edge_index = np.asarray(edge_index)
    W1 = np.asarray(W1, np.float32)
    b1 = np.asarray(b1, np.float32)
    W2 = np.asarray(W2, np.float32)
    b2 = np.asarray(b2, np.float32)

    per_core, deg, meta = _preprocess(edge_index)
    xt, deg_tbl, w2p, b2p = _host_tensors(x, deg, W1, b1, W2, b2)
    w1r = np.ascontiguousarray(W1.reshape(2, P, HID))
    b1r = b1.reshape(1, HID)
    if DT == BF16:
        xt = xt.astype(ml_dtypes.bfloat16)
        w1r = w1r.astype(ml_dtypes.bfloat16)
        w2p = w2p.astype(ml_dtypes.bfloat16)
        per_core = [(i16, dl.astype(ml_dtypes.bfloat16))
                    for (i16, dl) in per_core]

    cache_key = (meta["idx_cols"], meta["chunk_cols"],
                 meta["caps"].tobytes())
    nc = _CACHE.get(cache_key)
    if nc is None:
        nc = _build_program(meta)
        _CACHE[cache_key] = nc

    in_maps = []
    for c in range(NCORES):
        idx16, dstloc = per_core[c]
        in_maps.append({
            "xt": xt, "w1": w1r, "b1": b1r, "w2": w2p, "b2": b2p,
            "deg_tbl": deg_tbl, "deg_own": _deg_own(deg, c),
            "idx16": idx16, "dstloc": dstloc,
        })
    res = run_bass_kernel_spmd(nc, in_maps, list(range(NCORES)),
                               trace=_trace)
    out = np.empty((N, NCLS), np.float32)
    for c in range(NCORES):
        out[c * SHARD:(c + 1) * SHARD] = res.results[c]["out"][:SHARD]
    if _trace:
        kernel._last_exec_ns = res.exec_time_ns
        kernel._last_res = res
    return out
